# revision 3
# baseline (speedup 1.0000x reference)
"""Trainium2 Bass kernel for nn_AttentionHead (B=8, T=2048, D=1024, H=64).

Single attention head with additive relative-position scores:
    k = x@Wk + bk; q = x@Wq + bq; v = x@Wv
    S = (q k^T) sqrt(H) + einsum(btc,tvc->btv)(q, rel)  [+ causal mask]
    out = softmax(S) @ v

Distribution: query-block parallel over 8 NeuronCores. Core c owns query
blocks {c, 15-c} (128 rows each) so causal work is balanced. One SPMD
program runs on every core; per-core differences (which rel rows, which
causal mask, which q columns) are carried entirely by the input data.

Numerics: the PE's fp32 matmul path is only ~tf32 accurate, which is not
enough for the large-magnitude logits here (softmax near-ties amplify
score error). All score-path matmuls therefore run as bf16 hi/lo split
products (error ~2^-17); x, W and rel are split on the host. v and the
P@V reduction run in single bf16 (linear error only).
"""

import os

import numpy as np
import ml_dtypes

import concourse.bass as bass
import concourse.tile as tile
from concourse import bacc, mybir
from concourse.bass_utils import run_bass_kernel_spmd

BF16 = mybir.dt.bfloat16
F32 = mybir.dt.float32

# problem shape (hardcoded per contract)
B, T, D, H = 8, 2048, 1024, 64
TB = 128              # query-block rows
NBLK = T // TB        # 16
NCORES = 8
NEG = -1.0e9

LAST_EXEC_NS = None


def _cfg(causal: bool):
    # per-core uniform padded extents for the (small, big) block slots
    if causal:
        exts = (1024, 2048)
    else:
        exts = (2048, 2048)
    return {
        "B": B, "T": T, "D": D, "H": H, "TB": TB,
        "exts": exts, "smax": T,
    }


def build_nc(cfg):
    Bc, Tc, Dc, Hc, TBc = cfg["B"], cfg["T"], cfg["D"], cfg["H"], cfg["TB"]
    exts = cfg["exts"]
    smax = cfg["smax"]
    ND = Dc // 128                 # d-tiles
    NQ = 2 * TBc                   # own query rows (2 blocks)
    SCH = 512                      # s-chunk for projections / scores
    NPAIR = TBc // 2               # 64 t-pairs per block
    NGRP = NPAIR // 4              # 16 groups of 4 pairs

    nc = bacc.Bacc("TRN2", target_bir_lowering=False, debug=False,
                   num_devices=NCORES)

    # ---- I/O ----
    xh = nc.dram_tensor("xh", [Dc, Bc, Tc], BF16, kind="ExternalInput")
    xl = nc.dram_tensor("xl", [Dc, Bc, Tc], BF16, kind="ExternalInput")
    xqh = nc.dram_tensor("xqh", [Dc, 2, Bc, TBc], BF16, kind="ExternalInput")
    xql = nc.dram_tensor("xql", [Dc, 2, Bc, TBc], BF16, kind="ExternalInput")
    wkh = nc.dram_tensor("wkh", [Dc, Hc], BF16, kind="ExternalInput")
    wkl = nc.dram_tensor("wkl", [Dc, Hc], BF16, kind="ExternalInput")
    wqh = nc.dram_tensor("wqh", [Dc, Hc], BF16, kind="ExternalInput")
    wql = nc.dram_tensor("wql", [Dc, Hc], BF16, kind="ExternalInput")
    wv = nc.dram_tensor("wv", [Dc, Hc], BF16, kind="ExternalInput")
    bk8 = nc.dram_tensor("bk8", [Hc, 1], F32, kind="ExternalInput")
    bq_ = nc.dram_tensor("bq", [Hc, 1], F32, kind="ExternalInput")
    relh = nc.dram_tensor("relh", [2, TBc, Hc, Tc], BF16, kind="ExternalInput")
    rell = nc.dram_tensor("rell", [2, TBc, Hc, Tc], BF16, kind="ExternalInput")
    maskA = nc.dram_tensor("maskA", [TBc, exts[0]], F32, kind="ExternalInput")
    maskB = nc.dram_tensor("maskB", [TBc, exts[1]], F32, kind="ExternalInput")
    identf = nc.dram_tensor("identf", [128, 128], F32, kind="ExternalInput")
    identb = nc.dram_tensor("identb", [128, 128], BF16, kind="ExternalInput")
    out = nc.dram_tensor("out", [Bc, 2, TBc, Hc], F32, kind="ExternalOutput")

    NST = smax // 128              # s-tiles for V
    with tile.TileContext(nc) as tc:
        # ---------------- persistent tiles ----------------
        with (
            tc.tile_pool(name="persist", bufs=1) as pp,
            tc.tile_pool(name="weights", bufs=1) as pw,
        ):
            # k stacked: rows 0-63 = kT_hi, rows 64-127 = kT_lo ; cols (b, s)
            kstack = pp.tile([128, Bc * smax], BF16, tag="kstack")
            # q stacks: cols (blk, b, t)
            qmain = pp.tile([128, NQ * Bc], BF16, tag="qmain")   # hi top, lo bottom
            qcorr = pp.tile([128, NQ * Bc], BF16, tag="qcorr")   # lo top, hi bottom
            # V natural: [s-part, (b, stile, h)]
            vnat = pp.tile([128, Bc * NST * Hc], BF16, tag="vnat")
            mA = pp.tile([TBc, exts[0]], F32, tag="maskA")
            mB = pp.tile([TBc, exts[1]], F32, tag="maskB")
            idf = pw.tile([128, 128], F32, tag="identf")
            idb = pw.tile([128, 128], BF16, tag="identb")
            wk_t = pw.tile([128, ND, 2, Hc], BF16, tag="wk")     # (dtile, hi/lo, h)
            wq_t = pw.tile([128, ND, 2, Hc], BF16, tag="wq")
            wv_t = pw.tile([128, ND, Hc], BF16, tag="wv")
            bk_t = pw.tile([Hc, 1], F32, tag="bk")
            bq_t = pw.tile([Hc, 1], F32, tag="bq")

            nc.sync.dma_start(mA, maskA.ap())
            nc.sync.dma_start(mB, maskB.ap())
            nc.sync.dma_start(idf, identf.ap())
            nc.sync.dma_start(idb, identb.ap())
            nc.sync.dma_start(
                wk_t[:, :, 0, :], wkh.ap().rearrange("(n p) h -> p n h", p=128))
            nc.sync.dma_start(
                wk_t[:, :, 1, :], wkl.ap().rearrange("(n p) h -> p n h", p=128))
            nc.sync.dma_start(
                wq_t[:, :, 0, :], wqh.ap().rearrange("(n p) h -> p n h", p=128))
            nc.sync.dma_start(
                wq_t[:, :, 1, :], wql.ap().rearrange("(n p) h -> p n h", p=128))
            nc.sync.dma_start(
                wv_t, wv.ap().rearrange("(n p) h -> p n h", p=128))
            nc.sync.dma_start(bk_t, bk8.ap())
            nc.sync.dma_start(bq_t, bq_.ap())

            # ---------------- phase 1: projections ----------------
            with (
                tc.tile_pool(name="xstream", bufs=4) as px,
                tc.tile_pool(name="pstage", bufs=3) as pst,
                tc.tile_pool(name="psum1", bufs=2, space="PSUM") as pp1,
                tc.tile_pool(name="psumvt", bufs=2, space="PSUM") as ppvt,
            ):
                xf = xh.ap().rearrange("(n p) b t -> p n (b t)", p=128)
                xlf = xl.ap().rearrange("(n p) b t -> p n (b t)", p=128)
                for ci in range(Bc * smax // SCH):
                    c0 = ci * SCH
                    bidx = c0 // smax
                    s0 = c0 % smax
                    xht = px.tile([128, ND, SCH], BF16, tag="xh")
                    xlt = px.tile([128, ND, SCH], BF16, tag="xl")
                    nc.sync.dma_start(xht, xf[:, :, c0:c0 + SCH])
                    nc.sync.dma_start(xlt, xlf[:, :, c0:c0 + SCH])
                    # k projection: 3-pass hi/lo
                    psk = pp1.tile([Hc, SCH], F32, tag="pk")
                    for dt_ in range(ND):
                        nc.tensor.matmul(psk, wk_t[:, dt_, 0, :], xht[:, dt_],
                                         start=(dt_ == 0), stop=False)
                    for dt_ in range(ND):
                        nc.tensor.matmul(psk, wk_t[:, dt_, 1, :], xht[:, dt_],
                                         start=False, stop=False)
                    for dt_ in range(ND):
                        nc.tensor.matmul(psk, wk_t[:, dt_, 0, :], xlt[:, dt_],
                                         start=False, stop=(dt_ == ND - 1))
                    ktmp = pst.tile([Hc, SCH], F32, tag="ktmp")
                    nc.scalar.activation(ktmp, psk,
                                         mybir.ActivationFunctionType.Identity,
                                         bias=bk_t[:, :], scale=1.0)
                    nc.vector.tensor_copy(kstack[0:Hc, c0:c0 + SCH], ktmp)
                    nc.vector.tensor_tensor(
                        kstack[Hc:128, c0:c0 + SCH], ktmp,
                        kstack[0:Hc, c0:c0 + SCH], mybir.AluOpType.subtract)
                    # v projection: single-pass bf16, then transpose to natural
                    psv = pp1.tile([Hc, SCH], F32, tag="pv")
                    for dt_ in range(ND):
                        nc.tensor.matmul(psv, wv_t[:, dt_], xht[:, dt_],
                                         start=(dt_ == 0), stop=(dt_ == ND - 1))
                    vtmp = pst.tile([Hc, SCH], F32, tag="vtmp")
                    nc.any.tensor_copy(vtmp, psv)
                    for sub in range(SCH // 128):
                        pvt = ppvt.tile([128, Hc], F32, tag="pvt")
                        nc.tensor.transpose(
                            pvt, vtmp[:, sub * 128:(sub + 1) * 128],
                            idf[0:Hc, 0:Hc])
                        st = (s0 + sub * 128) // 128
                        nc.any.tensor_copy(vnat[:, bidx * NST * Hc + st * Hc:
                                                bidx * NST * Hc + (st + 1) * Hc],
                                           pvt)

                # q projection over own columns: cols (blk, b, t)
                xqf = xqh.ap().rearrange("(n p) k b t -> p n (k b t)", p=128)
                xqlf = xql.ap().rearrange("(n p) k b t -> p n (k b t)", p=128)
                for ci in range(2 * Bc * TBc // SCH):
                    c0 = ci * SCH
                    xht = px.tile([128, ND, SCH], BF16, tag="xh")
                    xlt = px.tile([128, ND, SCH], BF16, tag="xl")
                    nc.sync.dma_start(xht, xqf[:, :, c0:c0 + SCH])
                    nc.sync.dma_start(xlt, xqlf[:, :, c0:c0 + SCH])
                    psq = pp1.tile([Hc, SCH], F32, tag="pk")
                    for dt_ in range(ND):
                        nc.tensor.matmul(psq, wq_t[:, dt_, 0, :], xht[:, dt_],
                                         start=(dt_ == 0), stop=False)
                    for dt_ in range(ND):
                        nc.tensor.matmul(psq, wq_t[:, dt_, 1, :], xht[:, dt_],
                                         start=False, stop=False)
                    for dt_ in range(ND):
                        nc.tensor.matmul(psq, wq_t[:, dt_, 0, :], xlt[:, dt_],
                                         start=False, stop=(dt_ == ND - 1))
                    qtmp = pst.tile([Hc, SCH], F32, tag="ktmp")
                    nc.scalar.activation(qtmp, psq,
                                         mybir.ActivationFunctionType.Identity,
                                         bias=bq_t[:, :], scale=1.0)
                    nc.vector.tensor_copy(qmain[0:Hc, c0:c0 + SCH], qtmp)
                    nc.vector.tensor_tensor(
                        qmain[Hc:128, c0:c0 + SCH], qtmp,
                        qmain[0:Hc, c0:c0 + SCH], mybir.AluOpType.subtract)
                    nc.vector.tensor_copy(qcorr[Hc:128, c0:c0 + SCH],
                                          qmain[0:Hc, c0:c0 + SCH])
                    nc.vector.tensor_copy(qcorr[0:Hc, c0:c0 + SCH],
                                          qmain[Hc:128, c0:c0 + SCH])

            # ---------------- phase 2: per-block scores ----------------
            with (
                tc.tile_pool(name="bd", bufs=2) as pbd,
                tc.tile_pool(name="relstream", bufs=6) as prel,
                tc.tile_pool(name="stage", bufs=2) as pstage,
                tc.tile_pool(name="Spool", bufs=8) as pS,
                tc.tile_pool(name="Ppool", bufs=2) as pP,
                tc.tile_pool(name="PTpool", bufs=3) as pPT,
                tc.tile_pool(name="outpool", bufs=2) as po,
                tc.tile_pool(name="stats", bufs=4) as pstat,
                tc.tile_pool(name="psumrel", bufs=2, space="PSUM") as ppr,
                tc.tile_pool(name="psumS", bufs=2, space="PSUM") as ppS,
                tc.tile_pool(name="psumpt", bufs=2, space="PSUM") as pppt,
                tc.tile_pool(name="psumout", bufs=2, space="PSUM") as ppo,
            ):
                for blk in range(2):
                    ext = exts[blk]
                    nch = ext // SCH
                    msk = mA if blk == 0 else mB
                    # --- build block-diagonal q tiles (hi and lo) ---
                    bdh = pbd.tile([128, NPAIR * 16], BF16, tag="bdh")
                    bdl = pbd.tile([128, NPAIR * 16], BF16, tag="bdl")
                    nc.vector.memset(bdh, 0.0)
                    nc.vector.memset(bdl, 0.0)
                    qblk_h = (qmain[0:Hc, blk * Bc * TBc:(blk + 1) * Bc * TBc]
                              .rearrange("c (b t) -> c b t", b=Bc))
                    qblk_l = (qmain[Hc:128, blk * Bc * TBc:(blk + 1) * Bc * TBc]
                              .rearrange("c (b t) -> c b t", b=Bc))
                    for j in range(2):
                        dst_h = (bdh[j * Hc:(j + 1) * Hc]
                                 .rearrange("c (p s) -> c p s", s=16)
                                 [:, :, j * 8:j * 8 + 8])
                        src_h = qblk_h[:, :, j::2].rearrange("c b p -> c p b")
                        nc.vector.tensor_copy(dst_h, src_h)
                        dst_l = (bdl[j * Hc:(j + 1) * Hc]
                                 .rearrange("c (p s) -> c p s", s=16)
                                 [:, :, j * 8:j * 8 + 8])
                        src_l = qblk_l[:, :, j::2].rearrange("c b p -> c p b")
                        nc.vector.tensor_copy(dst_l, src_l)

                    # --- S tiles, pre-filled by the rel shuffle ---
                    S_b = [pS.tile([TBc, ext], F32, tag="S", name=f"S_{blk}_{i}")
                            for i in range(Bc)]

                    relf = relh.ap()
                    relfl = rell.ap()
                    for g in range(NGRP):
                        stg = pstage.tile([128, ext], F32, tag="stage")
                        for ch in range(nch):
                            v0 = ch * SCH
                            psr = ppr.tile([128, SCH], F32, tag="pr")
                            for u in range(4):
                                p = 4 * g + u
                                # rel tile [(j,c), v] for pair p
                                rht = prel.tile([128, SCH], BF16, tag="rh")
                                rlt = prel.tile([128, SCH], BF16, tag="rl")
                                src_h = (relf[blk, 2 * p:2 * p + 2, :,
                                              v0:v0 + SCH]
                                         .rearrange("j c v -> (j c) v"))
                                src_l = (relfl[blk, 2 * p:2 * p + 2, :,
                                               v0:v0 + SCH]
                                         .rearrange("j c v -> (j c) v"))
                                nc.sync.dma_start(rht, src_h)
                                nc.sync.dma_start(rlt, src_l)
                                pslice = psr[32 * u:32 * u + 16, :]
                                bd_h = bdh[:, p * 16:p * 16 + 16]
                                bd_l = bdl[:, p * 16:p * 16 + 16]
                                nc.tensor.matmul(pslice, bd_h, rht,
                                                 start=True, stop=False,
                                                 tile_position=(0, 32 * u))
                                nc.tensor.matmul(pslice, bd_l, rht,
                                                 start=False, stop=False,
                                                 tile_position=(0, 32 * u))
                                nc.tensor.matmul(pslice, bd_h, rlt,
                                                 start=False, stop=True,
                                                 tile_position=(0, 32 * u))
                            nc.any.tensor_copy(stg[:, v0:v0 + SCH], psr)
                        # scatter-shuffle rel scores into S rows
                        for j in range(2):
                            for b in range(Bc):
                                nc.sync.dma_start(
                                    S_b[b][8 * g + j:8 * g + 8:2, :],
                                    stg[8 * j + b::32, :])

                    # --- per-batch: qk scores, softmax, P^T, AV ---
                    for b in range(Bc):
                        S = S_b[b]
                        qm = qmain[0:Hc, (blk * Bc + b) * TBc:
                                   (blk * Bc + b + 1) * TBc]
                        qc = qcorr[:, (blk * Bc + b) * TBc:
                                   (blk * Bc + b + 1) * TBc]
                        for ch in range(nch):
                            s0 = ch * SCH
                            psS = ppS.tile([TBc, SCH], F32, tag="pS")
                            cols = slice(b * smax + s0, b * smax + s0 + SCH)
                            nc.tensor.matmul(psS, qm, kstack[0:Hc, cols],
                                             start=True, stop=False)
                            nc.tensor.matmul(psS, qc, kstack[:, cols],
                                             start=False, stop=True)
                            nc.vector.tensor_tensor(
                                S[:, s0:s0 + SCH], psS, S[:, s0:s0 + SCH],
                                mybir.AluOpType.add)
                            nc.vector.tensor_tensor(
                                S[:, s0:s0 + SCH], S[:, s0:s0 + SCH],
                                msk[:, s0:s0 + SCH], mybir.AluOpType.add)
                        negmax = pstat.tile([TBc, 1], F32, tag="negmax")
                        zsum = pstat.tile([TBc, 1], F32, tag="zsum")
                        rz = pstat.tile([TBc, 1], F32, tag="rz")
                        nc.vector.tensor_reduce(negmax, S,
                                                mybir.AxisListType.X,
                                                mybir.AluOpType.max,
                                                negate=True)
                        P = pP.tile([TBc, ext], BF16, tag="P")
                        nc.scalar.activation(P, S,
                                             mybir.ActivationFunctionType.Exp,
                                             bias=negmax[:, :], scale=1.0,
                                             accum_out=zsum[:, :])
                        nc.vector.reciprocal(rz, zsum)
                        pso = ppo.tile([TBc, Hc], F32, tag="po")
                        for st in range(ext // 128):
                            ppt = pppt.tile([128, 128], BF16, tag="pt")
                            nc.tensor.transpose(
                                ppt, P[:, st * 128:(st + 1) * 128], idb)
                            ptt = pPT.tile([128, 128], BF16, tag="ptt")
                            nc.any.tensor_copy(ptt, ppt)
                            nc.tensor.matmul(
                                pso, ptt,
                                vnat[:, (b * NST + st) * Hc:
                                     (b * NST + st + 1) * Hc],
                                start=(st == 0), stop=(st == ext // 128 - 1))
                        osb = po.tile([TBc, Hc], F32, tag="osb")
                        nc.vector.tensor_scalar_mul(osb, pso, rz[:, :])
                        nc.sync.dma_start(out.ap()[b, blk], osb)

    nc.compile()
    return nc


def _split(a):
    hi = np.asarray(a, dtype=np.float32).astype(ml_dtypes.bfloat16)
    lo = (np.asarray(a, dtype=np.float32) - hi.astype(np.float32)).astype(
        ml_dtypes.bfloat16)
    return hi, lo


def kernel(x, Wk, bk, Wq, bq, Wv, rel_pos_emb, mask, **_unused):
    global LAST_EXEC_NS
    x = np.asarray(x, dtype=np.float32)
    Wk = np.asarray(Wk, dtype=np.float32)
    bk = np.asarray(bk, dtype=np.float32)
    Wq = np.asarray(Wq, dtype=np.float32)
    bq = np.asarray(bq, dtype=np.float32)
    Wv = np.asarray(Wv, dtype=np.float32)
    rel = np.asarray(rel_pos_emb, dtype=np.float32)
    causal = bool(np.asarray(mask).item())
    cfg = _cfg(causal)
    exts = cfg["exts"]

    scale = np.float32(np.sqrt(H))
    # xT: [D, B, T]
    xT = np.ascontiguousarray(x.transpose(2, 0, 1))
    xh, xl = _split(xT)
    wkh, wkl = _split(Wk * scale)
    wqh, wql = _split(Wq)
    wvh = Wv.astype(ml_dtypes.bfloat16)
    bk8 = (bk * scale).reshape(H, 1).astype(np.float32)
    bqr = bq.reshape(H, 1).astype(np.float32)
    # relT: [T, H, T] (t, c, v)
    relT = np.ascontiguousarray(rel.transpose(0, 2, 1))
    rth, rtl = _split(relT)
    identf = np.eye(128, dtype=np.float32)
    identb = np.eye(128).astype(ml_dtypes.bfloat16)

    in_maps = []
    blocks = []
    for c in range(NCORES):
        bA, bB = c, NBLK - 1 - c
        blocks.append((bA, bB))
        relh_c = np.stack([rth[bA * TB:(bA + 1) * TB], rth[bB * TB:(bB + 1) * TB]])
        rell_c = np.stack([rtl[bA * TB:(bA + 1) * TB], rtl[bB * TB:(bB + 1) * TB]])
        xqh_c = np.stack([xh[:, :, bA * TB:(bA + 1) * TB],
                          xh[:, :, bB * TB:(bB + 1) * TB]], axis=1)
        xql_c = np.stack([xl[:, :, bA * TB:(bA + 1) * TB],
                          xl[:, :, bB * TB:(bB + 1) * TB]], axis=1)
        masks = []
        for slot, blkid in ((0, bA), (1, bB)):
            ext = exts[slot]
            t_idx = blkid * TB + np.arange(TB)[:, None]
            s_idx = np.arange(ext)[None, :]
            if causal:
                m = np.where(s_idx <= t_idx, 0.0, NEG)
            else:
                m = np.zeros((TB, ext))
            masks.append(np.ascontiguousarray(m, dtype=np.float32))
        in_maps.append({
            "xh": xh, "xl": xl,
            "xqh": np.ascontiguousarray(xqh_c),
            "xql": np.ascontiguousarray(xql_c),
            "wkh": wkh, "wkl": wkl, "wqh": wqh, "wql": wql, "wv": wvh,
            "bk8": bk8, "bq": bqr,
            "relh": np.ascontiguousarray(relh_c),
            "rell": np.ascontiguousarray(rell_c),
            "maskA": masks[0], "maskB": masks[1],
            "identf": identf, "identb": identb,
        })

    nc = build_nc(cfg)
    res = run_bass_kernel_spmd(nc, in_maps, core_ids=list(range(NCORES)))
    LAST_EXEC_NS = res.exec_time_ns
    if os.environ.get("KERNEL_TRACE") == "1":
        res_t = run_bass_kernel_spmd(
            nc, in_maps, core_ids=list(range(NCORES)), trace=True)
        LAST_EXEC_NS = res_t.exec_time_ns

    out = np.empty((B, T, H), dtype=np.float32)
    for c in range(NCORES):
        oc = res.results[c]["out"]          # [B, 2, TB, H]
        bA, bB = blocks[c]
        out[:, bA * TB:(bA + 1) * TB] = oc[:, 0]
        out[:, bB * TB:(bB + 1) * TB] = oc[:, 1]
    return out
